# revision 5
# baseline (speedup 1.0000x reference)
"""Trainium2 Bass kernel for the MoE-routing random-feature ridge problem.

Strategy (8 NeuronCores, feature-parallel):
  - Atoms are grouped by element (stable sort keeps molID order) and padded
    per element to a multiple of CH=512.  All cores process all atoms, but
    each core owns a 512-wide slice of the 4096 random features.
  - Per 512-atom chunk (single element e):
      PT   = reductors[e]^T @ gto_chunk^T          [256, 512]   (PE)
      PW   = PT^T @ W[e][:, fslice] + c[e,fslice]  [512, 512]   (PE, bias via
             a K=1 matmul row of ones against the remapped bias c)
      Fw   = range-wrap(PW) ; F = sin(Fw)          (DVE wrap, ACT sin)
      Z   += ST_chunk^T @ F                        (PE + DVE add)
    where c = wrap(b + pi/2) into (-pi, pi], so sin(x+c) = cos(x+b) and one
    DVE range-wrap covers |x| < 2*pi beyond the bias.
  - Z [1024 mols, 512 feats] is AllGathered (2 MB/rank) into the full
    Ztrain [1024, 4096]; each core then computes its 512-row slice of
    Z^T Z and Z^T Y with f32r matmuls.
  - Host applies scale^2 = 2/NFEAT (S entries are exact 1.0), adds
    lambda*I, and concatenates the slices.
"""

import sys

if "/opt/trn_rl_repo" not in sys.path:
    sys.path.insert(0, "/opt/trn_rl_repo")

import numpy as np

import concourse.bacc as bacc
import concourse.mybir as mybir
import concourse.tile as tile
from concourse import bass_utils

NCORES = 8
NATOMS = 16384
NMOL = 1024
REP = 512
PROJ = 256
NFEAT = 4096
NELEM = 4
LLAMBDA = 1e-6

CH = 512           # atoms per chunk
NF_LOC = NFEAT // NCORES   # features per core (512)
MOLT = NMOL // 128          # mol tiles (8)

F32 = mybir.dt.float32
F32R = mybir.dt.float32r

_cache = {}


def _plan(charges, molIDs):
    """Host-side chunking plan from charges/molIDs (static per compile)."""
    charges = np.asarray(charges)
    molIDs = np.asarray(molIDs)
    assert np.all(np.diff(molIDs) >= 0), "molIDs must be sorted"
    perm = np.argsort(charges, kind="stable")
    mol_p = molIDs[perm]
    chg_p = charges[perm]

    # padded element groups
    counts = np.bincount(charges, minlength=NELEM)
    padded = [int(np.ceil(c / CH) * CH) for c in counts]
    A_pad = int(sum(padded))
    n_chunks = A_pad // CH

    # index into permuted arrays for each padded slot (-1 = padding)
    slot_idx = np.full(A_pad, -1, dtype=np.int64)
    src_off = 0
    dst_off = 0
    for e in range(NELEM):
        c = int(counts[e])
        slot_idx[dst_off:dst_off + c] = np.arange(src_off, src_off + c)
        src_off += c
        dst_off += padded[e]

    chunk_elem = []
    chunk_m0 = []
    W_need = 1
    for c in range(n_chunks):
        sl = slot_idx[c * CH:(c + 1) * CH]
        real = sl >= 0
        if real.any():
            mols = mol_p[sl[real]]
            t_lo = int(mols.min()) // 128
            t_hi = int(mols.max()) // 128
            W_need = max(W_need, t_hi - t_lo + 1)
            chunk_m0.append(t_lo)
            e = int(chg_p[sl[real][0]])
        else:
            chunk_m0.append(0)
            e = int(np.searchsorted(np.cumsum(padded), c * CH, side="right"))
        chunk_elem.append(e)
    W = W_need
    chunk_m0 = [min(m0, MOLT - W) for m0 in chunk_m0]

    # nonzero (k-tile, wt) blocks of ST per chunk + ST data
    st_blocks = []   # list per chunk: list of (kt, wt) nonzero
    ST = np.zeros((n_chunks, CH, W * 128), dtype=np.float32)
    for c in range(n_chunks):
        sl = slot_idx[c * CH:(c + 1) * CH]
        real = np.nonzero(sl >= 0)[0]
        blocks = set()
        if len(real):
            ml = mol_p[sl[real]] - chunk_m0[c] * 128
            ok = (ml >= 0) & (ml < W * 128)
            ST[c, real[ok], ml[ok]] = 1.0
            for a, m in zip(real[ok], ml[ok]):
                blocks.add((int(a) // 128, int(m) // 128))
        st_blocks.append(sorted(blocks))

    return dict(perm=perm, slot_idx=slot_idx, A_pad=A_pad, n_chunks=n_chunks,
                chunk_elem=chunk_elem, chunk_m0=chunk_m0, W=W, ST=ST,
                st_blocks=st_blocks)


def _build(plan):
    n_chunks = plan["n_chunks"]
    W = plan["W"]
    chunk_elem = plan["chunk_elem"]
    chunk_m0 = plan["chunk_m0"]
    st_blocks = plan["st_blocks"]

    nc = bacc.Bacc(num_devices=NCORES)
    gto_d = nc.dram_tensor("gto_swz", [n_chunks, 128, 4 * CH], F32R, kind="ExternalInput")
    st_d = nc.dram_tensor("st_swz", [n_chunks, 128, 4 * W * 128], F32R, kind="ExternalInput")
    red_d = nc.dram_tensor("red_swz", [128, NELEM * 4 * PROJ], F32R, kind="ExternalInput")
    w_d = nc.dram_tensor("w_swz", [128, NELEM * 2 * NF_LOC], F32R, kind="ExternalInput")
    c_d = nc.dram_tensor("c_swz", [1, NELEM * NF_LOC], F32R, kind="ExternalInput")
    ones_d = nc.dram_tensor("ones", [1, 128], F32R, kind="ExternalInput")
    y_d = nc.dram_tensor("y_swz", [128, MOLT], F32, kind="ExternalInput")
    ztz_d = nc.dram_tensor("ztz", [NF_LOC, NFEAT], F32, kind="ExternalOutput")
    zty_d = nc.dram_tensor("zty", [NF_LOC, 1], F32, kind="ExternalOutput")

    with tile.TileContext(nc) as tc:
        with (
            tc.tile_pool(name="const", bufs=1) as constp,
            tc.tile_pool(name="zacc", bufs=1) as zaccp,
            tc.tile_pool(name="dram", bufs=1, space="DRAM") as dramp,
        ):
            red_sb = constp.tile([128, NELEM * 4 * PROJ], F32R, tag="red")
            w_sb = constp.tile([128, NELEM * 2 * NF_LOC], F32R, tag="w")
            c_sb = constp.tile([1, NELEM * NF_LOC], F32R, tag="c")
            ones_sb = constp.tile([1, 128], F32R, tag="ones")
            y_sb = constp.tile([128, MOLT], F32, tag="y")
            nc.sync.dma_start(out=red_sb[:], in_=red_d[:])
            nc.sync.dma_start(out=w_sb[:], in_=w_d[:])
            nc.sync.dma_start(out=c_sb[:], in_=c_d[:])
            nc.sync.dma_start(out=ones_sb[:], in_=ones_d[:])
            nc.sync.dma_start(out=y_sb[:], in_=y_d[:])

            z_sb = zaccp.tile([128, NMOL // 128 * NF_LOC], F32, tag="z")     # [128, 4096]
            zr_sb = zaccp.tile([128, NMOL // 128 * NF_LOC], F32R, tag="zr")
            nc.vector.memset(z_sb[:], 0.0)

            in_b = dramp.tile([NMOL, NF_LOC], F32R, tag="agin")
            ag_b = dramp.tile([NCORES * NMOL, NF_LOC], F32R, addr_space="Shared", tag="agout")

            # ---------------- phase 1: chunks ----------------
            with (
                tc.tile_pool(name="gtop", bufs=2) as gtop,
                tc.tile_pool(name="stp", bufs=2) as stp,
                tc.tile_pool(name="ptp", bufs=2) as ptp,
                tc.tile_pool(name="fp", bufs=2) as fpool,
                tc.tile_pool(name="ppt", bufs=2, space="PSUM") as ppt,
                tc.tile_pool(name="pf", bufs=3, space="PSUM") as pf,
                tc.tile_pool(name="pz", bufs=3, space="PSUM") as pz,
            ):
                for ci in range(n_chunks):
                    e = chunk_elem[ci]
                    m0 = chunk_m0[ci]
                    gto_t = gtop.tile([128, 4 * CH], F32R, tag="gto")
                    nc.sync.dma_start(out=gto_t[:], in_=gto_d[ci, :, :])
                    st_t = stp.tile([128, 4 * W * 128], F32R, tag="st")
                    if st_blocks[ci]:
                        nc.sync.dma_start(out=st_t[:], in_=st_d[ci, :, :])

                    # PT [256, 512] -> pt_sb [128, 2*512]
                    pt_sb = ptp.tile([128, 2 * CH], F32R, tag="pt")
                    for mp in range(2):  # proj tile
                        pt_ps = ppt.tile([128, CH], F32, tag="ptps")
                        for kt in range(4):  # rep k tile
                            nc.tensor.matmul(
                                pt_ps[:],
                                red_sb[:, (e * 4 + kt) * PROJ + mp * 128:
                                          (e * 4 + kt) * PROJ + mp * 128 + 128],
                                gto_t[:, kt * CH:(kt + 1) * CH],
                                start=(kt == 0), stop=(kt == 3),
                            )
                        nc.scalar.copy(pt_sb[:, mp * CH:(mp + 1) * CH], pt_ps[:])

                    # feats F [512 atoms, 512 feats] -> f_sb [128, 4*512]
                    f_sb = fpool.tile([128, 4 * NF_LOC], F32R, tag="f")
                    for mt in range(4):  # atom tile
                        f_ps = pf.tile([128, NF_LOC], F32, tag="fps")
                        for kp in range(2):  # proj k tile
                            nc.tensor.matmul(
                                f_ps[:],
                                pt_sb[:, kp * CH + mt * 128: kp * CH + mt * 128 + 128],
                                w_sb[:, (e * 2 + kp) * NF_LOC:(e * 2 + kp + 1) * NF_LOC],
                                start=(kp == 0), stop=False,
                            )
                        nc.tensor.matmul(
                            f_ps[:],
                            ones_sb[:],
                            c_sb[:, e * NF_LOC:(e + 1) * NF_LOC],
                            start=False, stop=True,
                        )
                        fw = fpool.tile([128, NF_LOC], F32, tag="fw")
                        nc.vector.add_range_wrap(
                            out=fw[:], in_=f_ps[:],
                            shift=0.0, bound=np.pi, period=2 * np.pi,
                        )
                        nc.scalar.activation(
                            f_sb[:, mt * NF_LOC:(mt + 1) * NF_LOC], fw[:],
                            mybir.ActivationFunctionType.Sin,
                        )

                    # Z += ST^T @ F per mol tile in window
                    for wt in range(W):
                        kts = [kt for (kt, w2) in st_blocks[ci] if w2 == wt]
                        if not kts:
                            continue
                        z_ps = pz.tile([128, NF_LOC], F32, tag="zps")
                        for i, kt in enumerate(kts):
                            nc.tensor.matmul(
                                z_ps[:],
                                st_t[:, (kt * W + wt) * 128:(kt * W + wt) * 128 + 128],
                                f_sb[:, kt * NF_LOC:(kt + 1) * NF_LOC],
                                start=(i == 0), stop=(i == len(kts) - 1),
                            )
                        mt_out = m0 + wt
                        nc.vector.tensor_add(
                            z_sb[:, mt_out * NF_LOC:(mt_out + 1) * NF_LOC],
                            z_sb[:, mt_out * NF_LOC:(mt_out + 1) * NF_LOC],
                            z_ps[:],
                        )

                # convert Z to f32r + ship to collective
                for k in range(MOLT):
                    nc.scalar.copy(
                        zr_sb[:, k * NF_LOC:(k + 1) * NF_LOC],
                        z_sb[:, k * NF_LOC:(k + 1) * NF_LOC],
                    )
                    nc.sync.dma_start(
                        out=in_b[k * 128:(k + 1) * 128, :],
                        in_=zr_sb[:, k * NF_LOC:(k + 1) * NF_LOC],
                    )
                nc.gpsimd.collective_compute(
                    "AllGather",
                    mybir.AluOpType.bypass,
                    replica_groups=[list(range(NCORES))],
                    ins=[in_b[:].opt()],
                    outs=[ag_b[:].opt()],
                )

            # ---------------- phase 2: ZTZ slice + ZtY ----------------
            with (
                tc.tile_pool(name="panel", bufs=4) as panelp,
                tc.tile_pool(name="osb", bufs=4) as osbp,
                tc.tile_pool(name="pztz", bufs=4, space="PSUM") as pztz,
                tc.tile_pool(name="pzty", bufs=1, space="PSUM") as pzty,
            ):
                zty_ps = pzty.tile([128, 4], F32, tag="ztyps")
                for m in range(4):
                    for k in range(MOLT):
                        nc.tensor.matmul(
                            zty_ps[:, m:m + 1],
                            z_sb[:, k * NF_LOC + m * 128: k * NF_LOC + m * 128 + 128],
                            y_sb[:, k:k + 1],
                            start=(k == 0), stop=(k == MOLT - 1),
                        )
                zty_sb = osbp.tile([128, 4], F32, tag="ztysb")
                nc.vector.tensor_copy(zty_sb[:], zty_ps[:])
                nc.sync.dma_start(
                    out=zty_d[:].rearrange("(m p) o -> p (m o)", p=128),
                    in_=zty_sb[:],
                )

                for n in range(NCORES):
                    ztz_ps = [pztz.tile([128, NF_LOC], F32, tag="ztzps", name=f"ztz_ps_{n}_{m}") for m in range(4)]
                    for k in range(MOLT):
                        pan = panelp.tile([128, NF_LOC], F32R, tag="pan")
                        nc.sync.dma_start(
                            out=pan[:],
                            in_=ag_b[n * NMOL + k * 128: n * NMOL + (k + 1) * 128, :],
                        )
                        for m in range(4):
                            nc.tensor.matmul(
                                ztz_ps[m][:],
                                zr_sb[:, k * NF_LOC + m * 128: k * NF_LOC + m * 128 + 128],
                                pan[:],
                                start=(k == 0), stop=(k == MOLT - 1),
                            )
                    for m in range(4):
                        o_sb = osbp.tile([128, NF_LOC], F32, tag="osb")
                        nc.scalar.copy(o_sb[:], ztz_ps[m][:])
                        nc.sync.dma_start(
                            out=ztz_d[m * 128:(m + 1) * 128, n * NF_LOC:(n + 1) * NF_LOC],
                            in_=o_sb[:],
                        )
    nc.finalize()
    return nc


def _prep_inputs(gto, reductors, W_in, b, Y, plan):
    n_chunks = plan["n_chunks"]
    A_pad = plan["A_pad"]
    slot_idx = plan["slot_idx"]
    W = plan["W"]

    gto_p = np.zeros((A_pad, REP), dtype=np.float32)
    real = slot_idx >= 0
    gto_p[real] = np.asarray(gto)[plan["perm"][slot_idx[real]]]
    # [A_pad, REP] -> [n_chunks, 128(rep-part), 4*CH]
    gto_swz = np.ascontiguousarray(
        gto_p.reshape(n_chunks, CH, 4, 128).transpose(0, 3, 2, 1)
    ).reshape(n_chunks, 128, 4 * CH)

    st_swz = np.ascontiguousarray(
        plan["ST"].reshape(n_chunks, 4, 128, W * 128).transpose(0, 2, 1, 3)
    ).reshape(n_chunks, 128, 4 * W * 128)

    red_swz = np.ascontiguousarray(
        np.asarray(reductors).reshape(NELEM, 4, 128, PROJ).transpose(2, 0, 1, 3)
    ).reshape(128, NELEM * 4 * PROJ)

    c_full = np.mod(np.asarray(b) + np.pi / 2 + np.pi, 2 * np.pi) - np.pi  # [-pi, pi)

    W_np = np.asarray(W_in)
    in_maps = []
    for d in range(NCORES):
        fsl = slice(d * NF_LOC, (d + 1) * NF_LOC)
        w_swz = np.ascontiguousarray(
            W_np[:, :, fsl].reshape(NELEM, 2, 128, NF_LOC).transpose(2, 0, 1, 3)
        ).reshape(128, NELEM * 2 * NF_LOC)
        c_swz = np.ascontiguousarray(c_full[:, fsl]).reshape(1, NELEM * NF_LOC)
        in_maps.append({
            "gto_swz": gto_swz,
            "st_swz": st_swz,
            "red_swz": red_swz,
            "w_swz": w_swz.astype(np.float32),
            "c_swz": c_swz.astype(np.float32),
            "ones": np.ones((1, 128), dtype=np.float32),
            "y_swz": np.ascontiguousarray(
                np.asarray(Y).reshape(MOLT, 128).T
            ).astype(np.float32),
        })
    return in_maps


def _get_built(charges, molIDs):
    key = (hash(np.asarray(charges).tobytes()), hash(np.asarray(molIDs).tobytes()))
    if key not in _cache:
        plan = _plan(charges, molIDs)
        nc = _build(plan)
        _cache[key] = (plan, nc)
    return _cache[key]


def run(gto, reductors, W, b, Y, charges, molIDs, trace=False, tmpdir=None):
    plan, nc = _get_built(charges, molIDs)
    in_maps = _prep_inputs(gto, reductors, W, b, Y, plan)
    res = bass_utils.run_bass_kernel_spmd(
        nc, in_maps, core_ids=list(range(NCORES)), trace=trace, tmpdir=tmpdir,
    )
    scale2 = 2.0 / NFEAT
    scale = np.float32(np.sqrt(scale2))
    ztz = np.concatenate([res.results[d]["ztz"] for d in range(NCORES)], axis=0)
    zty = np.concatenate([res.results[d]["zty"] for d in range(NCORES)], axis=0)
    ztz = ztz * np.float32(scale2)
    ztz[np.arange(NFEAT), np.arange(NFEAT)] += np.float32(LLAMBDA)
    zty = zty * scale
    out = np.concatenate([ztz, zty], axis=1).astype(np.float32)
    return out, res


def kernel(gto, reductors, W, b, Y, charges, molIDs):
    out, _ = run(gto, reductors, W, b, Y, charges, molIDs)
    return out


# revision 8
# speedup vs baseline: 1.1217x; 1.1217x over previous
"""Trainium2 Bass kernel for the MoE-routing random-feature ridge problem.

Strategy (8 NeuronCores, feature-parallel):
  - Atoms are grouped by element (stable sort keeps molID order) and padded
    per element to a multiple of CH=512.  All cores process all atoms, but
    each core owns a 512-wide slice of the 4096 random features.
  - Per 512-atom chunk (single element e):
      PT   = reductors[e]^T @ gto_chunk^T          [256, 512]   (PE)
      PW   = PT^T @ W[e][:, fslice] + c[e,fslice]  [512, 512]   (PE, bias via
             a K=1 matmul row of ones against the remapped bias c)
      Fw   = range-wrap(PW) ; F = sin(Fw)          (DVE wrap, ACT sin)
      Z   += ST_chunk^T @ F                        (PE + DVE add)
    where c = wrap(b + pi/2) into (-pi, pi], so sin(x+c) = cos(x+b) and one
    DVE range-wrap covers |x| < 2*pi beyond the bias.
  - Z [1024 mols, 512 feats] is AllGathered (2 MB/rank) into the full
    Ztrain [1024, 4096]; each core then computes its 512-row slice of
    Z^T Z and Z^T Y with f32r matmuls.
  - Host applies scale^2 = 2/NFEAT (S entries are exact 1.0), adds
    lambda*I, and concatenates the slices.
"""

import sys

if "/opt/trn_rl_repo" not in sys.path:
    sys.path.insert(0, "/opt/trn_rl_repo")

import numpy as np

import concourse.bacc as bacc
import concourse.mybir as mybir
import concourse.tile as tile
from concourse import bass_utils

NCORES = 8
NATOMS = 16384
NMOL = 1024
REP = 512
PROJ = 256
NFEAT = 4096
NELEM = 4
LLAMBDA = 1e-6

CH = 512           # atoms per chunk
NF_LOC = NFEAT // NCORES   # features per core (512)
MOLT = NMOL // 128          # mol tiles (8)

F32 = mybir.dt.float32
F32R = mybir.dt.float32r

_cache = {}


def _plan(charges, molIDs):
    """Host-side chunking plan from charges/molIDs (static per compile)."""
    charges = np.asarray(charges)
    molIDs = np.asarray(molIDs)
    assert np.all(np.diff(molIDs) >= 0), "molIDs must be sorted"
    perm = np.argsort(charges, kind="stable")
    mol_p = molIDs[perm]
    chg_p = charges[perm]

    # padded element groups
    counts = np.bincount(charges, minlength=NELEM)
    padded = [int(np.ceil(c / CH) * CH) for c in counts]
    A_pad = int(sum(padded))
    n_chunks = A_pad // CH

    # index into permuted arrays for each padded slot (-1 = padding)
    slot_idx = np.full(A_pad, -1, dtype=np.int64)
    src_off = 0
    dst_off = 0
    for e in range(NELEM):
        c = int(counts[e])
        slot_idx[dst_off:dst_off + c] = np.arange(src_off, src_off + c)
        src_off += c
        dst_off += padded[e]

    chunk_elem = []
    chunk_m0 = []
    W_need = 1
    for c in range(n_chunks):
        sl = slot_idx[c * CH:(c + 1) * CH]
        real = sl >= 0
        if real.any():
            mols = mol_p[sl[real]]
            t_lo = int(mols.min()) // 128
            t_hi = int(mols.max()) // 128
            W_need = max(W_need, t_hi - t_lo + 1)
            chunk_m0.append(t_lo)
            e = int(chg_p[sl[real][0]])
        else:
            chunk_m0.append(0)
            e = int(np.searchsorted(np.cumsum(padded), c * CH, side="right"))
        chunk_elem.append(e)
    W = W_need
    chunk_m0 = [min(m0, MOLT - W) for m0 in chunk_m0]

    # nonzero (k-tile, wt) blocks of ST per chunk + ST data
    st_blocks = []   # list per chunk: list of (kt, wt) nonzero
    ST = np.zeros((n_chunks, CH, W * 128), dtype=np.float32)
    for c in range(n_chunks):
        sl = slot_idx[c * CH:(c + 1) * CH]
        real = np.nonzero(sl >= 0)[0]
        blocks = set()
        if len(real):
            ml = mol_p[sl[real]] - chunk_m0[c] * 128
            ok = (ml >= 0) & (ml < W * 128)
            ST[c, real[ok], ml[ok]] = 1.0
            for a, m in zip(real[ok], ml[ok]):
                blocks.add((int(a) // 128, int(m) // 128))
        st_blocks.append(sorted(blocks))

    return dict(perm=perm, slot_idx=slot_idx, A_pad=A_pad, n_chunks=n_chunks,
                chunk_elem=chunk_elem, chunk_m0=chunk_m0, W=W, ST=ST,
                st_blocks=st_blocks)


def _build(plan):
    n_chunks = plan["n_chunks"]
    W = plan["W"]
    chunk_elem = plan["chunk_elem"]
    chunk_m0 = plan["chunk_m0"]
    st_blocks = plan["st_blocks"]

    nc = bacc.Bacc(num_devices=NCORES)
    gto_d = nc.dram_tensor("gto_swz", [n_chunks, 128, 4 * CH], F32R, kind="ExternalInput")
    st_d = nc.dram_tensor("st_swz", [n_chunks, 128, 4 * W * 128], F32R, kind="ExternalInput")
    red_d = nc.dram_tensor("red_swz", [128, NELEM * 4 * PROJ], F32R, kind="ExternalInput")
    w_d = nc.dram_tensor("w_swz", [128, NELEM * 2 * NF_LOC], F32R, kind="ExternalInput")
    c_d = nc.dram_tensor("c_swz", [1, NELEM * NF_LOC], F32R, kind="ExternalInput")
    ones_d = nc.dram_tensor("ones", [1, 128], F32R, kind="ExternalInput")
    y_d = nc.dram_tensor("y_swz", [128, MOLT], F32, kind="ExternalInput")
    ztz_d = nc.dram_tensor("ztz", [NF_LOC, NFEAT], F32, kind="ExternalOutput")
    zty_d = nc.dram_tensor("zty", [NF_LOC, 1], F32, kind="ExternalOutput")

    with tile.TileContext(nc) as tc:
        with (
            tc.tile_pool(name="const", bufs=1) as constp,
            tc.tile_pool(name="zacc", bufs=1) as zaccp,
            tc.tile_pool(name="dram", bufs=1, space="DRAM") as dramp,
        ):
            red_sb = constp.tile([128, NELEM * 4 * PROJ], F32R, tag="red")
            w_sb = constp.tile([128, NELEM * 2 * NF_LOC], F32R, tag="w")
            c_sb = constp.tile([1, NELEM * NF_LOC], F32R, tag="c")
            ones_sb = constp.tile([1, 128], F32R, tag="ones")
            y_sb = constp.tile([128, MOLT], F32, tag="y")
            nc.sync.dma_start(out=red_sb[:], in_=red_d[:])
            nc.sync.dma_start(out=w_sb[:], in_=w_d[:])
            nc.sync.dma_start(out=c_sb[:], in_=c_d[:])
            nc.sync.dma_start(out=ones_sb[:], in_=ones_d[:])
            nc.sync.dma_start(out=y_sb[:], in_=y_d[:])

            z_sb = zaccp.tile([128, NMOL // 128 * NF_LOC], F32, tag="z")     # [128, 4096]
            zr_sb = zaccp.tile([128, NMOL // 128 * NF_LOC], F32R, tag="zr")
            nc.vector.memset(z_sb[:], 0.0)

            in_b = dramp.tile([NMOL, NF_LOC], F32R, tag="agin")
            NG = 4                      # collective groups (2 mol tiles each)
            GM = MOLT // NG             # mol tiles per group
            ag_bs = [
                dramp.tile([NCORES * GM * 128, NF_LOC], F32R, addr_space="Shared",
                           tag=f"agout{g}", name=f"ag_b_{g}")
                for g in range(NG)
            ]

            # ---------------- phase 1: chunks (m0-sorted) ----------------
            order = sorted(range(n_chunks), key=lambda c: (chunk_m0[c], c))
            # last order-position touching each mol group
            group_last = [0] * NG
            for pos, ci in enumerate(order):
                if not st_blocks[ci]:
                    continue
                wts = {chunk_m0[ci] + wt for (kt, wt) in st_blocks[ci]}
                for mt in wts:
                    group_last[mt // GM] = max(group_last[mt // GM], pos)
            for g in range(NG):  # groups complete monotonically
                group_last[g] = max(group_last[:g + 1])
            group_at = {}
            for g in range(NG):
                group_at.setdefault(group_last[g], []).append(g)

            def emit_group_tail(g):
                for k in range(g * GM, (g + 1) * GM):
                    nc.scalar.copy(
                        zr_sb[:, k * NF_LOC:(k + 1) * NF_LOC],
                        z_sb[:, k * NF_LOC:(k + 1) * NF_LOC],
                    )
                    nc.sync.dma_start(
                        out=in_b[k * 128:(k + 1) * 128, :],
                        in_=zr_sb[:, k * NF_LOC:(k + 1) * NF_LOC],
                    )
                nc.gpsimd.collective_compute(
                    "AllGather",
                    mybir.AluOpType.bypass,
                    replica_groups=[list(range(NCORES))],
                    ins=[in_b[g * GM * 128:(g + 1) * GM * 128, :].opt()],
                    outs=[ag_bs[g][:].opt()],
                )

            with (
                tc.tile_pool(name="gtop", bufs=2) as gtop,
                tc.tile_pool(name="stp", bufs=2) as stp,
                tc.tile_pool(name="ptp", bufs=2) as ptp,
                tc.tile_pool(name="fp", bufs=2) as fpool,
                tc.tile_pool(name="ppt", bufs=2, space="PSUM") as ppt,
                tc.tile_pool(name="pf", bufs=2, space="PSUM") as pf,
                tc.tile_pool(name="pz", bufs=3, space="PSUM") as pz,
                tc.tile_pool(name="pzty", bufs=1, space="PSUM") as pzty,
            ):
                for pos, ci in enumerate(order):
                    e = chunk_elem[ci]
                    m0 = chunk_m0[ci]
                    gto_t = gtop.tile([128, 4 * CH], F32R, tag="gto")
                    nc.sync.dma_start(out=gto_t[:], in_=gto_d[ci, :, :])
                    st_t = stp.tile([128, 4 * W * 128], F32R, tag="st")
                    if st_blocks[ci]:
                        nc.sync.dma_start(out=st_t[:], in_=st_d[ci, :, :])

                    # PT [256, 512] -> pt_sb [128, 2*512]
                    pt_sb = ptp.tile([128, 2 * CH], F32R, tag="pt")
                    for mp in range(2):  # proj tile
                        pt_ps = ppt.tile([128, CH], F32, tag="ptps")
                        for kt in range(4):  # rep k tile
                            nc.tensor.matmul(
                                pt_ps[:],
                                red_sb[:, (e * 4 + kt) * PROJ + mp * 128:
                                          (e * 4 + kt) * PROJ + mp * 128 + 128],
                                gto_t[:, kt * CH:(kt + 1) * CH],
                                start=(kt == 0), stop=(kt == 3),
                            )
                        nc.scalar.copy(pt_sb[:, mp * CH:(mp + 1) * CH], pt_ps[:])

                    # feats F [512 atoms, 512 feats] -> f_sb [128, 4*512]
                    f_sb = fpool.tile([128, 4 * NF_LOC], F32R, tag="f")
                    for mt in range(4):  # atom tile
                        f_ps = pf.tile([128, NF_LOC], F32, tag="fps")
                        for kp in range(2):  # proj k tile
                            nc.tensor.matmul(
                                f_ps[:],
                                pt_sb[:, kp * CH + mt * 128: kp * CH + mt * 128 + 128],
                                w_sb[:, (e * 2 + kp) * NF_LOC:(e * 2 + kp + 1) * NF_LOC],
                                start=(kp == 0), stop=False,
                            )
                        nc.tensor.matmul(
                            f_ps[:],
                            ones_sb[:],
                            c_sb[:, e * NF_LOC:(e + 1) * NF_LOC],
                            start=False, stop=True,
                        )
                        fw = fpool.tile([128, NF_LOC], F32, tag="fw")
                        nc.vector.add_range_wrap(
                            out=fw[:], in_=f_ps[:],
                            shift=0.0, bound=np.pi, period=2 * np.pi,
                        )
                        nc.scalar.activation(
                            f_sb[:, mt * NF_LOC:(mt + 1) * NF_LOC], fw[:],
                            mybir.ActivationFunctionType.Sin,
                        )

                    # Z += ST^T @ F per mol tile in window
                    for wt in range(W):
                        kts = [kt for (kt, w2) in st_blocks[ci] if w2 == wt]
                        if not kts:
                            continue
                        z_ps = pz.tile([128, NF_LOC], F32, tag="zps")
                        for i, kt in enumerate(kts):
                            nc.tensor.matmul(
                                z_ps[:],
                                st_t[:, (kt * W + wt) * 128:(kt * W + wt) * 128 + 128],
                                f_sb[:, kt * NF_LOC:(kt + 1) * NF_LOC],
                                start=(i == 0), stop=(i == len(kts) - 1),
                            )
                        mt_out = m0 + wt
                        nc.vector.tensor_add(
                            z_sb[:, mt_out * NF_LOC:(mt_out + 1) * NF_LOC],
                            z_sb[:, mt_out * NF_LOC:(mt_out + 1) * NF_LOC],
                            z_ps[:],
                        )

                    if pos == n_chunks - 1:
                        # ZtY off the fp32 Z while the last gather flies
                        zty_ps = pzty.tile([128, 4], F32, tag="ztyps")
                        for m in range(4):
                            for k in range(MOLT):
                                nc.tensor.matmul(
                                    zty_ps[:, m:m + 1],
                                    z_sb[:, k * NF_LOC + m * 128: k * NF_LOC + m * 128 + 128],
                                    y_sb[:, k:k + 1],
                                    start=(k == 0), stop=(k == MOLT - 1),
                                )
                        zty_sb = zaccp.tile([128, 4], F32, tag="ztysb")
                        nc.vector.tensor_copy(zty_sb[:], zty_ps[:])
                        nc.sync.dma_start(
                            out=zty_d[:].rearrange("(m p) o -> p (m o)", p=128),
                            in_=zty_sb[:],
                        )

                    for g in group_at.get(pos, []):
                        emit_group_tail(g)

            # ---------------- phase 2: ZTZ slice ----------------
            with (
                tc.tile_pool(name="panel", bufs=8) as panelp,
                tc.tile_pool(name="osb", bufs=4) as osbp,
                tc.tile_pool(name="pztz", bufs=7, space="PSUM") as pztz,
            ):
                for n in range(NCORES):
                    ztz_ps = [pztz.tile([128, NF_LOC], F32, tag="ztzps", name=f"ztz_ps_{n}_{m}") for m in range(4)]
                    for k in range(MOLT):
                        g, kg = k // GM, k % GM
                        pan = panelp.tile([128, NF_LOC], F32R, tag="pan")
                        nc.sync.dma_start(
                            out=pan[:],
                            in_=ag_bs[g][(n * GM + kg) * 128:(n * GM + kg + 1) * 128, :],
                        )
                        for m in range(4):
                            nc.tensor.matmul(
                                ztz_ps[m][:],
                                zr_sb[:, k * NF_LOC + m * 128: k * NF_LOC + m * 128 + 128],
                                pan[:],
                                start=(k == 0), stop=(k == MOLT - 1),
                            )
                    for m in range(4):
                        o_sb = osbp.tile([128, NF_LOC], F32, tag="osb")
                        nc.scalar.copy(o_sb[:], ztz_ps[m][:])
                        nc.sync.dma_start(
                            out=ztz_d[m * 128:(m + 1) * 128, n * NF_LOC:(n + 1) * NF_LOC],
                            in_=o_sb[:],
                        )
    nc.finalize()
    return nc


def _prep_inputs(gto, reductors, W_in, b, Y, plan):
    n_chunks = plan["n_chunks"]
    A_pad = plan["A_pad"]
    slot_idx = plan["slot_idx"]
    W = plan["W"]

    gto_p = np.zeros((A_pad, REP), dtype=np.float32)
    real = slot_idx >= 0
    gto_p[real] = np.asarray(gto)[plan["perm"][slot_idx[real]]]
    # [A_pad, REP] -> [n_chunks, 128(rep-part), 4*CH]
    gto_swz = np.ascontiguousarray(
        gto_p.reshape(n_chunks, CH, 4, 128).transpose(0, 3, 2, 1)
    ).reshape(n_chunks, 128, 4 * CH)

    st_swz = np.ascontiguousarray(
        plan["ST"].reshape(n_chunks, 4, 128, W * 128).transpose(0, 2, 1, 3)
    ).reshape(n_chunks, 128, 4 * W * 128)

    red_swz = np.ascontiguousarray(
        np.asarray(reductors).reshape(NELEM, 4, 128, PROJ).transpose(2, 0, 1, 3)
    ).reshape(128, NELEM * 4 * PROJ)

    c_full = np.mod(np.asarray(b) + np.pi / 2 + np.pi, 2 * np.pi) - np.pi  # [-pi, pi)

    W_np = np.asarray(W_in)
    in_maps = []
    for d in range(NCORES):
        fsl = slice(d * NF_LOC, (d + 1) * NF_LOC)
        w_swz = np.ascontiguousarray(
            W_np[:, :, fsl].reshape(NELEM, 2, 128, NF_LOC).transpose(2, 0, 1, 3)
        ).reshape(128, NELEM * 2 * NF_LOC)
        c_swz = np.ascontiguousarray(c_full[:, fsl]).reshape(1, NELEM * NF_LOC)
        in_maps.append({
            "gto_swz": gto_swz,
            "st_swz": st_swz,
            "red_swz": red_swz,
            "w_swz": w_swz.astype(np.float32),
            "c_swz": c_swz.astype(np.float32),
            "ones": np.ones((1, 128), dtype=np.float32),
            "y_swz": np.ascontiguousarray(
                np.asarray(Y).reshape(MOLT, 128).T
            ).astype(np.float32),
        })
    return in_maps


def _get_built(charges, molIDs):
    key = (hash(np.asarray(charges).tobytes()), hash(np.asarray(molIDs).tobytes()))
    if key not in _cache:
        plan = _plan(charges, molIDs)
        nc = _build(plan)
        _cache[key] = (plan, nc)
    return _cache[key]


def run(gto, reductors, W, b, Y, charges, molIDs, trace=False, tmpdir=None):
    plan, nc = _get_built(charges, molIDs)
    in_maps = _prep_inputs(gto, reductors, W, b, Y, plan)
    res = bass_utils.run_bass_kernel_spmd(
        nc, in_maps, core_ids=list(range(NCORES)), trace=trace, tmpdir=tmpdir,
    )
    scale2 = 2.0 / NFEAT
    scale = np.float32(np.sqrt(scale2))
    ztz = np.concatenate([res.results[d]["ztz"] for d in range(NCORES)], axis=0)
    zty = np.concatenate([res.results[d]["zty"] for d in range(NCORES)], axis=0)
    ztz = ztz * np.float32(scale2)
    ztz[np.arange(NFEAT), np.arange(NFEAT)] += np.float32(LLAMBDA)
    zty = zty * scale
    out = np.concatenate([ztz, zty], axis=1).astype(np.float32)
    return out, res


def kernel(gto, reductors, W, b, Y, charges, molIDs):
    out, _ = run(gto, reductors, W, b, Y, charges, molIDs)
    return out


# revision 10
# speedup vs baseline: 1.2901x; 1.1501x over previous
"""Trainium2 Bass kernel for the MoE-routing random-feature ridge problem.

Strategy (8 NeuronCores, feature-parallel):
  - Atoms are grouped by element (stable sort keeps molID order) and padded
    per element to a multiple of CH=512.  All cores process all atoms, but
    each core owns a 512-wide slice of the 4096 random features.
  - Per 512-atom chunk (single element e):
      PT   = reductors[e]^T @ gto_chunk^T          [256, 512]   (PE)
      PW   = PT^T @ W[e][:, fslice] + c[e,fslice]  [512, 512]   (PE, bias via
             a K=1 matmul row of ones against the remapped bias c)
      Fw   = range-wrap(PW) ; F = sin(Fw)          (DVE wrap, ACT sin)
      Z   += ST_chunk^T @ F                        (PE + DVE add)
    where c = wrap(b + pi/2) into (-pi, pi], so sin(x+c) = cos(x+b) and one
    DVE range-wrap covers |x| < 2*pi beyond the bias.
  - Z [1024 mols, 512 feats] is AllGathered (2 MB/rank) into the full
    Ztrain [1024, 4096]; each core then computes its 512-row slice of
    Z^T Z and Z^T Y with f32r matmuls.
  - Host applies scale^2 = 2/NFEAT (S entries are exact 1.0), adds
    lambda*I, and concatenates the slices.
"""

import sys

if "/opt/trn_rl_repo" not in sys.path:
    sys.path.insert(0, "/opt/trn_rl_repo")

import numpy as np

import concourse.bacc as bacc
import concourse.mybir as mybir
import concourse.tile as tile
from concourse import bass_utils

NCORES = 8
NATOMS = 16384
NMOL = 1024
REP = 512
PROJ = 256
NFEAT = 4096
NELEM = 4
LLAMBDA = 1e-6

CH = 512           # atoms per chunk
NF_LOC = NFEAT // NCORES   # features per core (512)
MOLT = NMOL // 128          # mol tiles (8)

F32 = mybir.dt.float32
F32R = mybir.dt.float32r
BF16 = mybir.dt.bfloat16

_cache = {}


def _plan(charges, molIDs):
    """Host-side chunking plan from charges/molIDs (static per compile)."""
    charges = np.asarray(charges)
    molIDs = np.asarray(molIDs)
    assert np.all(np.diff(molIDs) >= 0), "molIDs must be sorted"
    perm = np.argsort(charges, kind="stable")
    mol_p = molIDs[perm]
    chg_p = charges[perm]

    # padded element groups
    counts = np.bincount(charges, minlength=NELEM)
    padded = [int(np.ceil(c / CH) * CH) for c in counts]
    A_pad = int(sum(padded))
    n_chunks = A_pad // CH

    # index into permuted arrays for each padded slot (-1 = padding)
    slot_idx = np.full(A_pad, -1, dtype=np.int64)
    src_off = 0
    dst_off = 0
    for e in range(NELEM):
        c = int(counts[e])
        slot_idx[dst_off:dst_off + c] = np.arange(src_off, src_off + c)
        src_off += c
        dst_off += padded[e]

    chunk_elem = []
    chunk_m0 = []
    W_need = 1
    for c in range(n_chunks):
        sl = slot_idx[c * CH:(c + 1) * CH]
        real = sl >= 0
        if real.any():
            mols = mol_p[sl[real]]
            t_lo = int(mols.min()) // 128
            t_hi = int(mols.max()) // 128
            W_need = max(W_need, t_hi - t_lo + 1)
            chunk_m0.append(t_lo)
            e = int(chg_p[sl[real][0]])
        else:
            chunk_m0.append(0)
            e = int(np.searchsorted(np.cumsum(padded), c * CH, side="right"))
        chunk_elem.append(e)
    W = W_need
    chunk_m0 = [min(m0, MOLT - W) for m0 in chunk_m0]

    # nonzero (k-tile, wt) blocks of ST per chunk + ST data
    st_blocks = []   # list per chunk: list of (kt, wt) nonzero
    ST = np.zeros((n_chunks, CH, W * 128), dtype=np.float32)
    for c in range(n_chunks):
        sl = slot_idx[c * CH:(c + 1) * CH]
        real = np.nonzero(sl >= 0)[0]
        blocks = set()
        if len(real):
            ml = mol_p[sl[real]] - chunk_m0[c] * 128
            ok = (ml >= 0) & (ml < W * 128)
            ST[c, real[ok], ml[ok]] = 1.0
            for a, m in zip(real[ok], ml[ok]):
                blocks.add((int(a) // 128, int(m) // 128))
        st_blocks.append(sorted(blocks))

    return dict(perm=perm, slot_idx=slot_idx, A_pad=A_pad, n_chunks=n_chunks,
                chunk_elem=chunk_elem, chunk_m0=chunk_m0, W=W, ST=ST,
                st_blocks=st_blocks)


def _build(plan):
    n_chunks = plan["n_chunks"]
    W = plan["W"]
    chunk_elem = plan["chunk_elem"]
    chunk_m0 = plan["chunk_m0"]
    st_blocks = plan["st_blocks"]

    nc = bacc.Bacc(num_devices=NCORES)
    gto_d = nc.dram_tensor("gto_swz", [n_chunks, 128, 4 * CH], F32R, kind="ExternalInput")
    st_d = nc.dram_tensor("st_swz", [n_chunks, 128, 4 * W * 128], BF16, kind="ExternalInput")
    red_d = nc.dram_tensor("red_swz", [128, NELEM * 4 * PROJ], F32R, kind="ExternalInput")
    w_d = nc.dram_tensor("w_swz", [128, NELEM * 2 * NF_LOC], F32R, kind="ExternalInput")
    c_d = nc.dram_tensor("c_swz", [1, NELEM * NF_LOC], F32R, kind="ExternalInput")
    ones_d = nc.dram_tensor("ones", [1, 128], F32R, kind="ExternalInput")
    y_d = nc.dram_tensor("y_swz", [128, MOLT], F32, kind="ExternalInput")
    ztz_d = nc.dram_tensor("ztz", [NF_LOC, NFEAT], F32, kind="ExternalOutput")
    zty_d = nc.dram_tensor("zty", [NF_LOC, 1], F32, kind="ExternalOutput")

    with tile.TileContext(nc) as tc:
        with (
            tc.tile_pool(name="const", bufs=1) as constp,
            tc.tile_pool(name="zacc", bufs=1) as zaccp,
            tc.tile_pool(name="dram", bufs=1, space="DRAM") as dramp,
        ):
            red_sb = constp.tile([128, NELEM * 4 * PROJ], F32R, tag="red")
            w_sb = constp.tile([128, NELEM * 2 * NF_LOC], F32R, tag="w")
            c_sb = constp.tile([1, NELEM * NF_LOC], F32R, tag="c")
            ones_sb = constp.tile([1, 128], F32R, tag="ones")
            y_sb = constp.tile([128, MOLT], F32, tag="y")
            for q in range(4):
                qs = NELEM * 4 * PROJ // 4
                nc.sync.dma_start(out=red_sb[:, q * qs:(q + 1) * qs],
                                  in_=red_d[:, q * qs:(q + 1) * qs])
                ws = NELEM * 2 * NF_LOC // 4
                nc.sync.dma_start(out=w_sb[:, q * ws:(q + 1) * ws],
                                  in_=w_d[:, q * ws:(q + 1) * ws])
            nc.sync.dma_start(out=c_sb[:], in_=c_d[:])
            nc.sync.dma_start(out=ones_sb[:], in_=ones_d[:])
            nc.sync.dma_start(out=y_sb[:], in_=y_d[:])

            z_sb = zaccp.tile([128, NMOL // 128 * NF_LOC], F32, tag="z")     # [128, 4096]
            zr_sb = zaccp.tile([128, NMOL // 128 * NF_LOC], BF16, tag="zr")
            nc.vector.memset(z_sb[:], 0.0)

            in_b = dramp.tile([NMOL, NF_LOC], BF16, tag="agin")
            NG = 4                      # collective groups (2 mol tiles each)
            GM = MOLT // NG             # mol tiles per group
            ag_bs = [
                dramp.tile([NCORES * GM * 128, NF_LOC], BF16, addr_space="Shared",
                           tag=f"agout{g}", name=f"ag_b_{g}")
                for g in range(NG)
            ]

            # ---------------- phase 1: chunks (m0-sorted) ----------------
            order = sorted(range(n_chunks), key=lambda c: (chunk_m0[c], c))
            # last order-position touching each mol group
            group_last = [0] * NG
            for pos, ci in enumerate(order):
                if not st_blocks[ci]:
                    continue
                wts = {chunk_m0[ci] + wt for (kt, wt) in st_blocks[ci]}
                for mt in wts:
                    group_last[mt // GM] = max(group_last[mt // GM], pos)
            for g in range(NG):  # groups complete monotonically
                group_last[g] = max(group_last[:g + 1])
            group_at = {}
            for g in range(NG):
                group_at.setdefault(group_last[g], []).append(g)

            def emit_group_tail(g):
                for k in range(g * GM, (g + 1) * GM):
                    nc.gpsimd.tensor_copy(
                        zr_sb[:, k * NF_LOC:(k + 1) * NF_LOC],
                        z_sb[:, k * NF_LOC:(k + 1) * NF_LOC],
                    )
                    nc.sync.dma_start(
                        out=in_b[k * 128:(k + 1) * 128, :],
                        in_=zr_sb[:, k * NF_LOC:(k + 1) * NF_LOC],
                    )
                nc.gpsimd.collective_compute(
                    "AllGather",
                    mybir.AluOpType.bypass,
                    replica_groups=[list(range(NCORES))],
                    ins=[in_b[g * GM * 128:(g + 1) * GM * 128, :].opt()],
                    outs=[ag_bs[g][:].opt()],
                )

            with (
                tc.tile_pool(name="gtop", bufs=2) as gtop,
                tc.tile_pool(name="stp", bufs=2) as stp,
                tc.tile_pool(name="ptp", bufs=2) as ptp,
                tc.tile_pool(name="fp", bufs=2) as fpool,
                tc.tile_pool(name="ppt", bufs=2, space="PSUM") as ppt,
                tc.tile_pool(name="pf", bufs=2, space="PSUM") as pf,
                tc.tile_pool(name="pz", bufs=3, space="PSUM") as pz,
                tc.tile_pool(name="pzty", bufs=1, space="PSUM") as pzty,
            ):
                for pos, ci in enumerate(order):
                    e = chunk_elem[ci]
                    m0 = chunk_m0[ci]
                    gto_t = gtop.tile([128, 4 * CH], F32R, tag="gto")
                    nc.sync.dma_start(out=gto_t[:], in_=gto_d[ci, :, :])
                    st_t = stp.tile([128, 4 * W * 128], BF16, tag="st")
                    if st_blocks[ci]:
                        nc.sync.dma_start(out=st_t[:], in_=st_d[ci, :, :])

                    # PT [256, 512] -> pt_sb [128, 2*512]
                    pt_sb = ptp.tile([128, 2 * CH], F32R, tag="pt")
                    for mp in range(2):  # proj tile
                        pt_ps = ppt.tile([128, CH], F32, tag="ptps")
                        for kt in range(4):  # rep k tile
                            nc.tensor.matmul(
                                pt_ps[:],
                                red_sb[:, (e * 4 + kt) * PROJ + mp * 128:
                                          (e * 4 + kt) * PROJ + mp * 128 + 128],
                                gto_t[:, kt * CH:(kt + 1) * CH],
                                start=(kt == 0), stop=(kt == 3),
                            )
                        nc.scalar.copy(pt_sb[:, mp * CH:(mp + 1) * CH], pt_ps[:])

                    # feats F [512 atoms, 512 feats] -> f_sb [128, 4*512]
                    f_sb = fpool.tile([128, 4 * NF_LOC], BF16, tag="f")
                    for mt in range(4):  # atom tile
                        f_ps = pf.tile([128, NF_LOC], F32, tag="fps")
                        for kp in range(2):  # proj k tile
                            nc.tensor.matmul(
                                f_ps[:],
                                pt_sb[:, kp * CH + mt * 128: kp * CH + mt * 128 + 128],
                                w_sb[:, (e * 2 + kp) * NF_LOC:(e * 2 + kp + 1) * NF_LOC],
                                start=(kp == 0), stop=False,
                            )
                        nc.tensor.matmul(
                            f_ps[:],
                            ones_sb[:],
                            c_sb[:, e * NF_LOC:(e + 1) * NF_LOC],
                            start=False, stop=True,
                        )
                        fw = fpool.tile([128, NF_LOC], F32, tag="fw")
                        nc.vector.add_range_wrap(
                            out=fw[:], in_=f_ps[:],
                            shift=0.0, bound=np.pi, period=2 * np.pi,
                        )
                        nc.scalar.activation(
                            f_sb[:, mt * NF_LOC:(mt + 1) * NF_LOC], fw[:],
                            mybir.ActivationFunctionType.Sin,
                        )

                    # Z += ST^T @ F per mol tile in window
                    for wt in range(W):
                        kts = [kt for (kt, w2) in st_blocks[ci] if w2 == wt]
                        if not kts:
                            continue
                        z_ps = pz.tile([128, NF_LOC], F32, tag="zps")
                        for i, kt in enumerate(kts):
                            nc.tensor.matmul(
                                z_ps[:],
                                st_t[:, (kt * W + wt) * 128:(kt * W + wt) * 128 + 128],
                                f_sb[:, kt * NF_LOC:(kt + 1) * NF_LOC],
                                start=(i == 0), stop=(i == len(kts) - 1),
                            )
                        mt_out = m0 + wt
                        nc.vector.tensor_add(
                            z_sb[:, mt_out * NF_LOC:(mt_out + 1) * NF_LOC],
                            z_sb[:, mt_out * NF_LOC:(mt_out + 1) * NF_LOC],
                            z_ps[:],
                        )

                    if pos == n_chunks - 1:
                        # ZtY off the fp32 Z while the last gather flies
                        zty_ps = pzty.tile([128, 4], F32, tag="ztyps")
                        for m in range(4):
                            for k in range(MOLT):
                                nc.tensor.matmul(
                                    zty_ps[:, m:m + 1],
                                    z_sb[:, k * NF_LOC + m * 128: k * NF_LOC + m * 128 + 128],
                                    y_sb[:, k:k + 1],
                                    start=(k == 0), stop=(k == MOLT - 1),
                                )
                        zty_sb = zaccp.tile([128, 4], F32, tag="ztysb")
                        nc.vector.tensor_copy(zty_sb[:], zty_ps[:])
                        nc.sync.dma_start(
                            out=zty_d[:].rearrange("(m p) o -> p (m o)", p=128),
                            in_=zty_sb[:],
                        )

                    for g in group_at.get(pos, []):
                        emit_group_tail(g)

            # ---------------- phase 2: ZTZ slice ----------------
            with (
                tc.tile_pool(name="panel", bufs=8) as panelp,
                tc.tile_pool(name="osb", bufs=4) as osbp,
                tc.tile_pool(name="pztz", bufs=7, space="PSUM") as pztz,
            ):
                for n in range(NCORES):
                    ztz_ps = [pztz.tile([128, NF_LOC], F32, tag="ztzps", name=f"ztz_ps_{n}_{m}") for m in range(4)]
                    for k in range(MOLT):
                        g, kg = k // GM, k % GM
                        pan = panelp.tile([128, NF_LOC], BF16, tag="pan")
                        nc.sync.dma_start(
                            out=pan[:],
                            in_=ag_bs[g][(n * GM + kg) * 128:(n * GM + kg + 1) * 128, :],
                        )
                        for m in range(4):
                            nc.tensor.matmul(
                                ztz_ps[m][:],
                                zr_sb[:, k * NF_LOC + m * 128: k * NF_LOC + m * 128 + 128],
                                pan[:],
                                start=(k == 0), stop=(k == MOLT - 1),
                            )
                    for m in range(4):
                        o_sb = osbp.tile([128, NF_LOC], F32, tag="osb")
                        nc.scalar.copy(o_sb[:], ztz_ps[m][:])
                        nc.sync.dma_start(
                            out=ztz_d[m * 128:(m + 1) * 128, n * NF_LOC:(n + 1) * NF_LOC],
                            in_=o_sb[:],
                        )
    nc.finalize()
    return nc


def _prep_inputs(gto, reductors, W_in, b, Y, plan):
    n_chunks = plan["n_chunks"]
    A_pad = plan["A_pad"]
    slot_idx = plan["slot_idx"]
    W = plan["W"]

    gto_p = np.zeros((A_pad, REP), dtype=np.float32)
    real = slot_idx >= 0
    gto_p[real] = np.asarray(gto)[plan["perm"][slot_idx[real]]]
    # [A_pad, REP] -> [n_chunks, 128(rep-part), 4*CH]
    gto_swz = np.ascontiguousarray(
        gto_p.reshape(n_chunks, CH, 4, 128).transpose(0, 3, 2, 1)
    ).reshape(n_chunks, 128, 4 * CH)

    st_swz = np.ascontiguousarray(
        plan["ST"].reshape(n_chunks, 4, 128, W * 128).transpose(0, 2, 1, 3)
    ).reshape(n_chunks, 128, 4 * W * 128).astype(mybir.dt.np(BF16))

    red_swz = np.ascontiguousarray(
        np.asarray(reductors).reshape(NELEM, 4, 128, PROJ).transpose(2, 0, 1, 3)
    ).reshape(128, NELEM * 4 * PROJ)

    c_full = np.mod(np.asarray(b) + np.pi / 2 + np.pi, 2 * np.pi) - np.pi  # [-pi, pi)

    W_np = np.asarray(W_in)
    in_maps = []
    for d in range(NCORES):
        fsl = slice(d * NF_LOC, (d + 1) * NF_LOC)
        w_swz = np.ascontiguousarray(
            W_np[:, :, fsl].reshape(NELEM, 2, 128, NF_LOC).transpose(2, 0, 1, 3)
        ).reshape(128, NELEM * 2 * NF_LOC)
        c_swz = np.ascontiguousarray(c_full[:, fsl]).reshape(1, NELEM * NF_LOC)
        in_maps.append({
            "gto_swz": gto_swz,
            "st_swz": st_swz,
            "red_swz": red_swz,
            "w_swz": w_swz.astype(np.float32),
            "c_swz": c_swz.astype(np.float32),
            "ones": np.ones((1, 128), dtype=np.float32),
            "y_swz": np.ascontiguousarray(
                np.asarray(Y).reshape(MOLT, 128).T
            ).astype(np.float32),
        })
    return in_maps


def _get_built(charges, molIDs):
    key = (hash(np.asarray(charges).tobytes()), hash(np.asarray(molIDs).tobytes()))
    if key not in _cache:
        plan = _plan(charges, molIDs)
        nc = _build(plan)
        _cache[key] = (plan, nc)
    return _cache[key]


def run(gto, reductors, W, b, Y, charges, molIDs, trace=False, tmpdir=None):
    plan, nc = _get_built(charges, molIDs)
    in_maps = _prep_inputs(gto, reductors, W, b, Y, plan)
    res = bass_utils.run_bass_kernel_spmd(
        nc, in_maps, core_ids=list(range(NCORES)), trace=trace, tmpdir=tmpdir,
    )
    scale2 = 2.0 / NFEAT
    scale = np.float32(np.sqrt(scale2))
    ztz = np.concatenate([res.results[d]["ztz"] for d in range(NCORES)], axis=0)
    zty = np.concatenate([res.results[d]["zty"] for d in range(NCORES)], axis=0)
    ztz = ztz * np.float32(scale2)
    ztz[np.arange(NFEAT), np.arange(NFEAT)] += np.float32(LLAMBDA)
    zty = zty * scale
    out = np.concatenate([ztz, zty], axis=1).astype(np.float32)
    return out, res


def kernel(gto, reductors, W, b, Y, charges, molIDs):
    out, _ = run(gto, reductors, W, b, Y, charges, molIDs)
    return out
